# revision 6
# baseline (speedup 1.0000x reference)
"""Trainium2 Bass kernel for nn_Convolutionv2106Custom (gnn_message_passing).

Strategy: sort edges by destination node; shard contiguous 128-node blocks
across 8 cores balanced by edge count (dst-sharding => no collective needed:
each core owns its output rows). Per core, fixed SPMD structure of
NB=13 blocks x TPB=18 tiles x 128 edges (padded; pad edges have
edge_scalars=0 -> w=0 and dst_local=-1 -> zero one-hot column).

Per 128-edge tile on device (all matmuls float32r):
  FC1  h = silu(sT @ w1')          (feature-major h [64, 256], 2 tiles/mm)
  FC2  w = h.T @ w2'               (edge-major w [128e, 256] PSUM)
  TP   11 fused DVE scalar_tensor_tensor ops -> feat [128e, 384]
  SEG  one-hot(dst_local) matmul accumulated into per-block PSUM [128n, 384]
Node-feature gather via gpsimd dma_gather (node table split 64+96 cols so
row bytes are 256/384, both %256). All normalization constants folded into
w1'/w2' on the host.
"""

import math
import numpy as np

import bass_rust
import concourse.bass as bass
import concourse.mybir as mybir
from concourse import tile as _tile
from concourse.tile import TileContext
from concourse.vector_clock import ScopedClock
from concourse.bass import IndirectOffsetOnAxis

# ---------------------------------------------------------------- constants
N_NODES = 12500
N_EDGES = 200000
MUL0, MUL1 = 64, 32
NODE_DIM = 160
FC_IN, FC_HID = 16, 64
W_NUMEL = 192

NB_TOTAL = (N_NODES + 127) // 128          # 98 blocks of 128 nodes
NB = 13                                    # block slots per core
TPB = 18                                   # 128-edge tiles per block
TILE_E = 128
BLK_E = TPB * TILE_E                       # 2304 padded edges per block
E_PAD = NB * BLK_E                         # 29952 edge slots per core
N_CORES = 8
IDX_COLS = BLK_E // 16                     # 144 idx cols per block

F32 = mybir.dt.float32
F32R = mybir.dt.float32r
AOP = mybir.AluOpType
AFT = mybir.ActivationFunctionType


def _silu_norm():
    z = np.linspace(-12.0, 12.0, 200001)
    pdf = np.exp(-0.5 * z * z) / np.sqrt(2.0 * np.pi)
    silu = z / (1.0 + np.exp(-z))
    return np.float32(1.0 / np.sqrt(np.trapezoid(silu**2 * pdf, z)))


# ------------------------------------------------- tile tail-drain wait fix
# This walrus build rejects >1 sync wait on CTRL-type instructions; chunk the
# Tile tail-drain waits across single-wait no-ops.
def _chunked_drain_and_barrier(self, tick_clock, wait_clock):
    nc = self.nc
    drain_inst = nc.sync.drain()
    wait_clock.add_sem_waits(
        drain_inst.ins, ScopedClock({None: tick_clock.global_clock})
    )
    si = drain_inst.ins.sync_info
    if si is not None and len(si.on_wait) > 1:
        waits = list(si.on_wait)
        drain_inst.ins.sync_info = bass_rust.SyncInfo(
            on_wait=[], on_update=list(si.on_update)
        )
        for i in range(len(waits)):
            w = nc.sync.nop(nofuse=True, hint="tail_wait")
            w.ins.sync_info = bass_rust.SyncInfo(
                on_wait=waits[i : i + 1], on_update=[]
            )
    nc.all_engine_barrier()
    assert self.sems is not None
    popped = nc._tile_sem_poison_stack.pop()
    assert popped is self._sem_poison
    nc.clear_and_free_semaphores(list(self.sems.allocated().values()))
    nc.all_engine_barrier()


_tile.TileContext._drain_and_barrier = _chunked_drain_and_barrier


def _split_excess_waits(nc, max_waits: int = 1):
    """Walrus in this env caps sync waits per instruction; hoist overflow
    waits onto single-wait EventSemaphore carriers just before the
    instruction on the same engine."""
    n = 0
    for fn in nc.m.functions:
        for bb in fn.blocks:
            new = []
            for inst in bb.instructions:
                si = inst.sync_info
                if si is not None and len(si.on_wait) > max_waits:
                    waits = list(si.on_wait)
                    for i, w in enumerate(waits[: len(waits) - max_waits]):
                        ev = mybir.InstEventSemaphore(
                            name=f"{inst.name}_xw{i}", ins=[], outs=[])
                        ev.engine = inst.engine
                        ev.sync_info = bass_rust.SyncInfo(
                            on_wait=[w], on_update=[])
                        new.append(ev)
                        n += 1
                    inst.sync_info = bass_rust.SyncInfo(
                        on_wait=waits[len(waits) - max_waits:],
                        on_update=list(si.on_update))
                new.append(inst)
            bb.instructions = new
    return n


# ------------------------------------------------------------ device kernel
def _build_nc(repeat: int = 1) -> bass.Bass:
    nc = bass.Bass("TRN2", target_bir_lowering=False, debug=False)

    nodeT = nc.dram_tensor("nodeT", [N_NODES, 192], F32, kind="ExternalInput")
    sT_d = nc.dram_tensor("sT", [16, E_PAD], F32R, kind="ExternalInput")
    yw_d = nc.dram_tensor("yw", [128, NB * TPB * 4], F32, kind="ExternalInput")
    dw_d = nc.dram_tensor("dw", [128, NB * TPB], F32, kind="ExternalInput")
    sidx_d = nc.dram_tensor("sidx", [128, NB * TPB], mybir.dt.int32,
                            kind="ExternalInput")
    w1_d = nc.dram_tensor("w1s", [16, 64], F32R, kind="ExternalInput")
    w2_d = nc.dram_tensor("w2s", [64, 256], F32R, kind="ExternalInput")
    iota_d = nc.dram_tensor("iotaw", [128, 128], F32, kind="ExternalInput")
    out_d = nc.dram_tensor("out", [NB * 128, 384], F32, kind="ExternalOutput")

    with TileContext(nc) as tc:
        with (
            tc.tile_pool(name="const", bufs=1) as cpool,
            tc.tile_pool(name="x0p", bufs=2) as x0p,
            tc.tile_pool(name="x1p", bufs=2) as x1p,
            tc.tile_pool(name="stp", bufs=2) as stp,
            tc.tile_pool(name="hsb", bufs=3) as hsbp,
            tc.tile_pool(name="feat", bufs=4) as fpool,
            tc.tile_pool(name="oh", bufs=4) as ohpool,
            tc.tile_pool(name="osb", bufs=2) as opool,
            tc.tile_pool(name="tmp", bufs=4) as tpool,
            tc.tile_pool(name="hps", bufs=2, space="PSUM") as hpsp,
            tc.tile_pool(name="wps", bufs=4, space="PSUM") as wpsp,
            tc.tile_pool(name="bps", bufs=2, space="PSUM") as bpsp,
        ):
            w1s = cpool.tile([16, 64], F32R)
            nc.sync.dma_start(w1s[:], w1_d[:])
            w2s = cpool.tile([64, 256], F32R)
            nc.sync.dma_start(w2s[:], w2_d[:])
            iota = cpool.tile([128, 128], F32)
            nc.sync.dma_start(iota[:], iota_d[:])
            yw = cpool.tile([128, NB * TPB * 4], F32)
            nc.sync.dma_start(yw[:], yw_d[:])
            dw = cpool.tile([128, NB * TPB], F32)
            nc.sync.dma_start(dw[:], dw_d[:])
            sidx = cpool.tile([128, NB * TPB], mybir.dt.int32)
            nc.sync.dma_start(sidx[:], sidx_d[:])

            for _rep in range(repeat):
                for b in range(NB):
                    xg = x0p.tile([128, TPB, 192], F32, tag="xg")
                    for tg in range(TPB):
                        nc.gpsimd.indirect_dma_start(
                            xg[:, tg, :], None, nodeT[:],
                            IndirectOffsetOnAxis(
                                ap=sidx[:, b * TPB + tg:b * TPB + tg + 1],
                                axis=0),
                        )
                    sT = stp.tile([16, BLK_E], F32R, tag="sT")
                    nc.sync.dma_start(sT[:], sT_d[:, b * BLK_E:(b + 1) * BLK_E])

                    bps = bpsp.tile([128, 384], F32, tag="bps")
                    hsb = None
                    for t in range(TPB):
                        bt = b * TPB + t
                        if t % 2 == 0:
                            hps = hpsp.tile([64, 256], F32, tag="hps")
                            nc.tensor.matmul(
                                hps[:],
                                w1s[:],
                                sT[:, t * 128:(t + 2) * 128],
                                start=True, stop=True,
                            )
                            hsb = hsbp.tile([64, 256], F32R, tag="hsb")
                            nc.scalar.activation(hsb[:], hps[:], AFT.Silu)
                        wps = wpsp.tile([128, 256], F32, tag="wps")
                        nc.tensor.matmul(
                            wps[:],
                            hsb[:, (t % 2) * 128:(t % 2) * 128 + 128],
                            w2s[:],
                            start=True, stop=True,
                        )
                        w_a = wps[:, 0:64]
                        w_b = wps[:, 64:128]
                        w_c = wps[:, 128:160]
                        w_d = wps[:, 160:192]

                        x0t = xg[:, t, 0:64]
                        x1v = xg[:, t, 64:160].rearrange("p (u m) -> p u m", m=3)
                        y0 = yw[:, bt * 4:bt * 4 + 1]

                        F = fpool.tile([128, 384], F32R, tag="feat")
                        Fb = F[:, 96:288].rearrange("p (u m) -> p u m", m=3)
                        Fc = F[:, 288:384].rearrange("p (u m) -> p u m", m=3)

                        # path a: out_a = (x0 * y0) * w_a
                        nc.vector.scalar_tensor_tensor(
                            F[:, 0:64], x0t, y0, w_a, AOP.mult, AOP.mult)
                        # path b: out_b[:,u,m] = (x0*y1_m) * w_b
                        for m in range(3):
                            y1m = yw[:, bt * 4 + 1 + m:bt * 4 + 2 + m]
                            nc.vector.scalar_tensor_tensor(
                                Fb[:, :, m], x0t, y1m, w_b, AOP.mult, AOP.mult)
                        # path c: out_c[:,u,m] = (x1_m*y0) * w_c
                        for m in range(3):
                            nc.vector.scalar_tensor_tensor(
                                Fc[:, :, m], x1v[:, :, m], y0, w_c,
                                AOP.mult, AOP.mult)
                        # path d: out_d = (sum_m x1_m*y1_m) * w_d
                        tmp0 = tpool.tile([128, 32], F32, tag="d0")
                        nc.vector.tensor_scalar(
                            tmp0[:], x1v[:, :, 0],
                            yw[:, bt * 4 + 1:bt * 4 + 2], None, AOP.mult)
                        tmp1 = tpool.tile([128, 32], F32, tag="d1")
                        nc.vector.scalar_tensor_tensor(
                            tmp1[:], x1v[:, :, 1],
                            yw[:, bt * 4 + 2:bt * 4 + 3], tmp0[:],
                            AOP.mult, AOP.add)
                        tmp2 = tpool.tile([128, 32], F32, tag="d2")
                        nc.vector.scalar_tensor_tensor(
                            tmp2[:], x1v[:, :, 2],
                            yw[:, bt * 4 + 3:bt * 4 + 4], tmp1[:],
                            AOP.mult, AOP.add)
                        nc.vector.scalar_tensor_tensor(
                            F[:, 64:96], tmp2[:], 1.0, w_d, AOP.mult, AOP.mult)

                        # one-hot over the 128-node window, then accumulate
                        oh = ohpool.tile([128, 128], F32R, tag="oh")
                        nc.vector.tensor_scalar(
                            oh[:], iota[:], dw[:, bt:bt + 1], None, AOP.is_equal)
                        nc.tensor.matmul(
                            bps[:], oh[:], F[:],
                            start=(t == 0), stop=(t == TPB - 1),
                        )

                    osb = opool.tile([128, 384], F32, tag="osb")
                    nc.vector.tensor_copy(osb[:], bps[:])
                    nc.sync.dma_start(out_d[b * 128:(b + 1) * 128, :], osb[:])

    _split_excess_waits(nc)
    return nc


# -------------------------------------------------------------- host packing
def _pack(inputs):
    src = np.asarray(inputs["edge_src"]).astype(np.int64).ravel()
    dst = np.asarray(inputs["edge_dst"]).astype(np.int64).ravel()
    scal = np.asarray(inputs["edge_scalars"], dtype=np.float32)
    attr = np.asarray(inputs["edge_attr"], dtype=np.float32)
    E = src.shape[0]

    order = np.argsort(dst, kind="stable")
    src_s, dst_s = src[order], dst[order]
    scal_s, attr_s = scal[order], attr[order]
    blk = dst_s // 128
    counts = np.bincount(blk, minlength=NB_TOTAL).astype(np.int64)
    cum = np.concatenate([[0], np.cumsum(counts)])

    # contiguous block ranges per core, balanced by edge count, <= NB blocks
    cuts = [0]
    for c in range(1, N_CORES):
        ideal = E * c / N_CORES
        b1 = int(np.searchsorted(cum, ideal))
        b1 = max(b1, cuts[-1] + 1, NB_TOTAL - (N_CORES - c) * NB)
        b1 = min(b1, cuts[-1] + NB, NB_TOTAL - (N_CORES - c))
        cuts.append(b1)
    cuts.append(NB_TOTAL)

    in_maps, metas = [], []
    for c in range(N_CORES):
        g0, g1 = cuts[c], cuts[c + 1]
        nblk = g1 - g0
        assert 0 < nblk <= NB, (c, g0, g1)
        sidxw = np.zeros((128, NB * TPB), np.int32)
        sTw = np.zeros((16, E_PAD), np.float32)
        ywv = np.zeros((128, NB * TPB * 4), np.float32)
        dwv = np.full((128, NB * TPB), -1.0, np.float32)
        for b in range(nblk):
            g = g0 + b
            lo, hi = int(cum[g]), int(cum[g + 1])
            n = hi - lo
            assert n <= BLK_E, f"block {g} has {n} edges > {BLK_E}"
            idxp = np.zeros(BLK_E, np.int64)
            idxp[:n] = src_s[lo:hi]
            sidxw[:, b * TPB:(b + 1) * TPB] = \
                idxp.reshape(TPB, 128).T.astype(np.int32)
            sTw[:, b * BLK_E:b * BLK_E + n] = scal_s[lo:hi].T
            a = np.zeros((BLK_E, 4), np.float32)
            a[:n] = attr_s[lo:hi]
            ywv[:, b * TPB * 4:(b + 1) * TPB * 4] = \
                a.reshape(TPB, 128, 4).transpose(1, 0, 2).reshape(128, TPB * 4)
            dv = np.full(BLK_E, -1.0, np.float32)
            dv[:n] = (dst_s[lo:hi] - g * 128).astype(np.float32)
            dwv[:, b * TPB:(b + 1) * TPB] = dv.reshape(TPB, 128).T
        in_maps.append({
            "sT": sTw, "yw": ywv, "dw": dwv,
            "sidx": sidxw,
        })
        metas.append((g0, g1))
    return in_maps, metas


def _shared_inputs(inputs):
    node = np.ascontiguousarray(np.asarray(inputs["node_input"], np.float32))
    fc_w1 = np.asarray(inputs["fc_w1"], np.float32)
    fc_w2 = np.asarray(inputs["fc_w2"], np.float32)
    sn = _silu_norm()
    w1s = (fc_w1 / np.sqrt(np.float32(FC_IN))).astype(np.float32)
    w2s = np.zeros((64, 256), np.float32)
    w2s[:, :192] = fc_w2 * (sn / np.sqrt(np.float32(FC_HID)) / 4.0)
    w2s[:, 160:192] *= np.float32(1.0 / math.sqrt(3.0))
    iw = np.broadcast_to(np.arange(128, dtype=np.float32), (128, 128)).copy()
    nodeT = np.zeros((N_NODES, 192), np.float32)
    nodeT[:, :160] = node
    return {"nodeT": nodeT, "w1s": w1s, "w2s": w2s, "iotaw": iw}


def _assemble(results, metas):
    out = np.zeros((NB_TOTAL * 128, 384), np.float32)
    for c in range(N_CORES):
        g0, g1 = metas[c]
        oc = results[c]["out"]
        out[g0 * 128:g1 * 128] = oc[: (g1 - g0) * 128]
    return out[:N_NODES]


_CACHED = {}


def _get_runner(repeat: int = 1):
    if repeat not in _CACHED:
        from concourse.bass_utils import run_bass_kernel_spmd  # noqa: F401
        nc = _build_nc(repeat)
        _CACHED[repeat] = nc
    return _CACHED[repeat]


def kernel(**inputs) -> np.ndarray:
    from concourse.bass_utils import run_bass_kernel_spmd

    nc = _get_runner(1)
    shared = _shared_inputs(inputs)
    in_maps, metas = _pack(inputs)
    for m in in_maps:
        m.update(shared)
    res = run_bass_kernel_spmd(nc, in_maps, core_ids=list(range(N_CORES)))
    return _assemble(res.results, metas)
